# revision 3
# baseline (speedup 1.0000x reference)
"""Trainium2 Bass kernel for a ragged-sequence LSTM (nn_CH_LSTM).

Problem (hardcoded): T=512, B=64, DIN=1024, D=1024.
  c_init = broadcast(c0); h_init = tanh(c_init)
  per step t:  x = [x_t, h];  i,f,g,o = acts(x @ W_* + b_*)
               c = f*c + i*g;  h = o*tanh(c);  h[t >= len] = 0
  output: all h, [T, B, D] f32.

Device strategy (8 NeuronCores, data-parallel over batch: 8 seqs/core):
  phase 1: xw = X @ W_x as one large matmul; X arrives row-major bf16 and is
           transposed on-chip via the PE (identity matmul); xw -> DRAM bf16.
  phase 2: 512 sequential steps; per gate, xw_t + bias injected into PSUM via
           a tiny identity matmul, then h @ W_h accumulated with h^T-stationary
           matmuls (W_h resident in SBUF). State h/c fp32; output stored f16.

Wall-clock strategy (the axon tunnel moves ~50-80 MB/s, so bytes dominate):
  * per call only X (67MB bf16), mask+state init (<1MB) go up; out (67MB f16)
    comes down.  Weights upload once: 16MB sharded + on-fabric all_gather.
  * output zero-buffers are created on device (donated), never uploaded.
  * host passes are single-pass fused cast+copy (container has 1 CPU).
"""

import os
import sys

if "/opt/trn_rl_repo" not in sys.path:
    sys.path.insert(0, "/opt/trn_rl_repo")

import numpy as np
import ml_dtypes

T, B, DIN, D = 512, 64, 1024, 1024
NCORES = 8
BL = B // NCORES          # 8 sequences per core
G4 = 4 * D                # 4096 gate columns, order [i, g, f, o]
KD = D // 128             # 8 contraction tiles for the recurrent matmul
KX = DIN // 128           # 8 contraction tiles for the x matmul
TBL = T * BL              # 4096 flattened (t, b) rows per core
MT = TBL // 128           # 32 row tiles in phase 1
BF16 = ml_dtypes.bfloat16

_CACHE = {}


def _build_bass():
    import concourse.bass as bass
    import concourse.bacc as bacc
    import concourse.mybir as mybir
    from concourse import tile

    fp32 = mybir.dt.float32
    bf16 = mybir.dt.bfloat16
    f16 = mybir.dt.float16
    AF = mybir.ActivationFunctionType
    ALU = mybir.AluOpType
    ds = bass.ds

    nc = bacc.Bacc(trn_type="TRN2")

    x_d = nc.dram_tensor("x", [MT * 128, DIN], bf16, kind="ExternalInput")
    wx_d = nc.dram_tensor("wx", [KX, 128, G4], bf16, kind="ExternalInput")
    wh_d = nc.dram_tensor("wh", [KD, 128, G4], bf16, kind="ExternalInput")
    bias_d = nc.dram_tensor("biasrow", [1, G4], bf16, kind="ExternalInput")
    inj_d = nc.dram_tensor("inj", [BL + 1, BL], bf16, kind="ExternalInput")
    id_d = nc.dram_tensor("ident", [BL, BL], fp32, kind="ExternalInput")
    id128_d = nc.dram_tensor("id128", [128, 128], bf16, kind="ExternalInput")
    mask_d = nc.dram_tensor("mask", [BL, T], fp32, kind="ExternalInput")
    hc_d = nc.dram_tensor("hcinit", [2, BL, D], fp32, kind="ExternalInput")
    xw_d = nc.dram_tensor("xwbuf", [TBL, G4], bf16, kind="Internal")
    out_d = nc.dram_tensor("out", [TBL, D], f16, kind="ExternalOutput")

    with tile.TileContext(nc) as tc:
        with (
            tc.tile_pool(name="w", bufs=1) as wpool,
            tc.tile_pool(name="state", bufs=1) as spool,
            tc.tile_pool(name="gates", bufs=1) as gpool,
            tc.tile_pool(name="xwb", bufs=1) as xwbpool,
            tc.tile_pool(name="misc", bufs=1) as mpool,
            tc.tile_pool(name="o16", bufs=2) as o16pool,
        ):
            w_sb = wpool.tile([128, KD * G4], bf16)        # Wx in ph1, Wh in ph2
            h_sb = spool.tile([BL, D], fp32, tag="h")
            c_sb = spool.tile([BL, D], fp32, tag="c")
            mask_sb = mpool.tile([BL, T], fp32, tag="mask")
            inj_sb = mpool.tile([BL + 1, BL], bf16, tag="inj")
            id_sb = mpool.tile([BL, BL], fp32, tag="id")
            id128_sb = mpool.tile([128, 128], bf16, tag="id128")
            bias_sb = mpool.tile([1, G4], bf16, tag="bias")
            hT_sb = mpool.tile([128, KD * BL], bf16, tag="hT")
            xwb_A = xwbpool.tile([BL + 1, G4], bf16, tag="xa")
            xwb_B = xwbpool.tile([BL + 1, G4], bf16, tag="xb")
            i_sb = gpool.tile([BL, D], fp32, tag="gi")
            g_sb = gpool.tile([BL, D], fp32, tag="gg")
            f_sb = gpool.tile([BL, D], fp32, tag="gf")
            o_sb = gpool.tile([BL, D], fp32, tag="go")
            ig_sb = gpool.tile([BL, D], fp32, tag="ig")
            tanh_sb = gpool.tile([BL, D], fp32, tag="tc")

            nc.sync.dma_start(mask_sb[:], mask_d[:])
            nc.sync.dma_start(inj_sb[:], inj_d[:])
            nc.sync.dma_start(id_sb[:], id_d[:])
            nc.sync.dma_start(id128_sb[:], id128_d[:])
            nc.sync.dma_start(bias_sb[:], bias_d[:])
            nc.sync.dma_start(h_sb[:], hc_d[0])
            nc.sync.dma_start(c_sb[:], hc_d[1])
            nc.sync.dma_start(xwb_A[BL : BL + 1, :], bias_d[:])
            nc.sync.dma_start(xwb_B[BL : BL + 1, :], bias_d[:])
            for k in range(KX):
                nc.sync.dma_start(w_sb[:, k * G4 : (k + 1) * G4], wx_d[k])

            # ---------------- phase 1: xw = X @ Wx ----------------
            # X arrives row-major [row, din]; transpose 128x128 blocks on-chip.
            with (
                tc.tile_pool(name="ps1", bufs=2, space="PSUM") as ps1pool,
                tc.tile_pool(name="psT", bufs=1, space="PSUM") as psTpool,
                tc.tile_pool(name="xr", bufs=2) as xrpool,
                tc.tile_pool(name="xT", bufs=2) as xTpool,
                tc.tile_pool(name="xwo", bufs=3) as xwopool,
            ):
                with tc.For_i(0, MT, 1) as mv:
                    xr_sb = xrpool.tile([128, DIN], bf16, tag="xr")
                    nc.sync.dma_start(xr_sb[:], x_d[ds(mv * 128, 128), :])
                    psT = psTpool.tile([128, KX * 128], bf16, tag="psT")
                    for k in range(KX):
                        nc.tensor.transpose(
                            psT[:, k * 128 : (k + 1) * 128],
                            xr_sb[:, k * 128 : (k + 1) * 128],
                            id128_sb[:],
                        )
                    xT_sb = xTpool.tile([128, KX * 128], bf16, tag="xT")
                    nc.vector.tensor_copy(xT_sb[:], psT[:])
                    for q in range(4):
                        ps = ps1pool.tile([128, 1024], fp32, tag="ps")
                        for k in range(KX):
                            for n in range(2):
                                col = q * 1024 + n * 512
                                nc.tensor.matmul(
                                    ps[:, n * 512 : (n + 1) * 512],
                                    xT_sb[:, k * 128 : (k + 1) * 128],
                                    w_sb[:, k * G4 + col : k * G4 + col + 512],
                                    start=(k == 0),
                                    stop=(k == KX - 1),
                                )
                        xo = xwopool.tile([128, 1024], bf16, tag="xo")
                        nc.vector.tensor_copy(xo[:], ps[:])
                        nc.sync.dma_start(
                            xw_d[ds(mv * 128, 128), q * 1024 : (q + 1) * 1024],
                            xo[:],
                        )

            # ---------------- phase 2: recurrence ----------------
            for k in range(KD):
                nc.sync.dma_start(w_sb[:, k * G4 : (k + 1) * G4], wh_d[k])

            gate_specs = [
                (i_sb, AF.Sigmoid),
                (g_sb, AF.Tanh),
                (f_sb, AF.Sigmoid),
                (o_sb, AF.Sigmoid),
            ]

            with (
                tc.tile_pool(name="ps2", bufs=3, space="PSUM") as gps,
                tc.tile_pool(name="psT2", bufs=1, space="PSUM") as tps,
            ):
                def emit_step(t0, toff, xwb):
                    # h^T (bf16) for this step's stationary operands
                    hps = tps.tile([128, KD * BL], fp32, tag="ht")
                    for k in range(KD):
                        nc.tensor.transpose(
                            hps[:, k * BL : (k + 1) * BL],
                            h_sb[:, k * 128 : (k + 1) * 128],
                            id_sb[:],
                        )
                    nc.vector.tensor_copy(hT_sb[:], hps[:])

                    for gi, (gsb, func) in enumerate(gate_specs):
                        ps = gps.tile([BL, D], fp32, tag="g")
                        gcol = gi * D
                        for hh in range(2):
                            c0 = gcol + hh * 512
                            nc.tensor.matmul(
                                ps[:, hh * 512 : (hh + 1) * 512],
                                inj_sb[:],
                                xwb[:, c0 : c0 + 512],
                                start=True,
                                stop=False,
                            )
                        for k in range(KD):
                            for hh in range(2):
                                c0 = k * G4 + gcol + hh * 512
                                nc.tensor.matmul(
                                    ps[:, hh * 512 : (hh + 1) * 512],
                                    hT_sb[:, k * BL : (k + 1) * BL],
                                    w_sb[:, c0 : c0 + 512],
                                    start=False,
                                    stop=(k == KD - 1),
                                )
                        nc.scalar.activation(gsb[:], ps[:], func)

                    nc.vector.tensor_mul(ig_sb[:], i_sb[:], g_sb[:])
                    nc.vector.tensor_mul(c_sb[:], c_sb[:], f_sb[:])
                    nc.vector.tensor_add(c_sb[:], c_sb[:], ig_sb[:])
                    nc.scalar.activation(tanh_sb[:], c_sb[:], AF.Tanh)
                    tmask = mask_sb[:, ds(t0 + toff, 1)]
                    nc.vector.scalar_tensor_tensor(
                        h_sb[:], tanh_sb[:], tmask, o_sb[:],
                        ALU.mult, ALU.mult,
                    )
                    out16 = o16pool.tile([BL, D], f16, tag="o16")
                    nc.vector.tensor_copy(out16[:], h_sb[:])
                    nc.sync.dma_start(
                        out_d[ds(t0 * BL + toff * BL, BL), :], out16[:]
                    )

                with tc.For_i(0, T, 2) as t0:
                    nc.sync.dma_start(xwb_A[0:BL, :], xw_d[ds(t0 * BL, BL), :])
                    nc.sync.dma_start(
                        xwb_B[0:BL, :], xw_d[ds(t0 * BL + BL, BL), :]
                    )
                    emit_step(t0, 0, xwb_A)
                    emit_step(t0, 1, xwb_B)

    nc.finalize()
    return nc


def _runtime():
    """Build (once) the bass module + jitted device functions."""
    if "rt" in _CACHE:
        return _CACHE["rt"]

    import jax

    try:
        jax.config.update("jax_compilation_cache_dir", "/tmp/jax_cc_cache")
        jax.config.update("jax_persistent_cache_min_entry_size_bytes", -1)
        jax.config.update("jax_persistent_cache_min_compile_time_secs", 0)
    except Exception:
        pass

    import jax.numpy as jnp
    from jax.sharding import Mesh, PartitionSpec, NamedSharding
    from jax.experimental.shard_map import shard_map
    from concourse import mybir
    from concourse.bass2jax import (
        _bass_exec_p,
        install_neuronx_cc_hook,
        partition_id_tensor,
    )

    install_neuronx_cc_hook()
    nc = _build_bass()

    partition_name = (
        nc.partition_id_tensor.name if nc.partition_id_tensor else None
    )
    in_names, out_names, out_avals = [], [], []
    for alloc in nc.m.functions[0].allocations:
        if not isinstance(alloc, mybir.MemoryLocationSet):
            continue
        name = alloc.memorylocations[0].name
        if alloc.kind == "ExternalInput":
            if name != partition_name:
                in_names.append(name)
        elif alloc.kind == "ExternalOutput":
            out_names.append(name)
            shape = tuple(alloc.tensor_shape)
            dtype = mybir.dt.np(alloc.dtype)
            out_avals.append(jax.core.ShapedArray(shape, dtype))
    n_params = len(in_names)
    n_outs = len(out_avals)
    all_in_names = in_names + out_names
    if partition_name is not None:
        all_in_names.append(partition_name)

    def _body(*args):
        operands = list(args)
        if partition_name is not None:
            operands.append(partition_id_tensor())
        outs = _bass_exec_p.bind(
            *operands,
            out_avals=tuple(out_avals),
            in_names=tuple(all_in_names),
            out_names=tuple(out_names),
            lowering_input_output_aliases=(),
            sim_require_finite=True,
            sim_require_nnan=True,
            nc=nc,
        )
        return tuple(outs)

    devices = jax.devices()[:NCORES]
    mesh = Mesh(np.asarray(devices), ("core",))
    P = PartitionSpec
    shard = NamedSharding(mesh, P("core"))
    donate = tuple(range(n_params, n_params + n_outs))
    sharded = jax.jit(
        shard_map(
            _body,
            mesh=mesh,
            in_specs=(P("core"),) * (n_params + n_outs),
            out_specs=(P("core"),) * n_outs,
            check_rep=False,
        ),
        donate_argnums=donate,
        keep_unused=True,
    )

    zeros_fn = jax.jit(
        lambda: tuple(
            jnp.zeros((NCORES * a.shape[0], *a.shape[1:]), a.dtype)
            for a in out_avals
        ),
        out_shardings=tuple(shard for _ in out_avals),
    )

    # on-fabric broadcast: upload [n,128,G4] sharded, all_gather -> each core
    # holds the full array; global layout [8n,128,G4] with shard = full array.
    bcast_fn = jax.jit(
        shard_map(
            lambda a, b: (
                jax.lax.all_gather(a, "core", axis=0, tiled=True),
                jax.lax.all_gather(b, "core", axis=0, tiled=True),
            ),
            mesh=mesh,
            in_specs=(P("core"), P("core")),
            out_specs=(P("core"), P("core")),
        )
    )

    rt = {
        "jax": jax,
        "nc": nc,
        "in_names": in_names,
        "out_names": out_names,
        "sharded": sharded,
        "zeros_fn": zeros_fn,
        "bcast_fn": bcast_fn,
        "shard": shard,
        "mesh": mesh,
    }
    _CACHE["rt"] = rt
    return rt


def _rep8(arr):
    """Host-replicate a small array to global [8*s0, ...]."""
    return np.ascontiguousarray(
        np.broadcast_to(arr[None], (NCORES,) + arr.shape).reshape(
            (NCORES * arr.shape[0],) + arr.shape[1:]
        )
    )


def _stage_weights(rt, W_i_w, W_i_b, W_f_w, W_f_b, W_c_w, W_c_b, W_o_w, W_o_b):
    key = tuple(id(a) for a in (W_i_w, W_i_b, W_f_w, W_f_b, W_c_w, W_c_b,
                                W_o_w, W_o_b))
    st = _CACHE.get("weights")
    if st is not None and st["key"] == key:
        return st
    jax = rt["jax"]
    # gate order [i, g, f, o]
    Wcat = np.concatenate([W_i_w, W_c_w, W_f_w, W_o_w], axis=1).astype(BF16)
    bias = np.concatenate([W_i_b, W_c_b, W_f_b, W_o_b]).astype(BF16)
    wx = np.ascontiguousarray(Wcat[:DIN].reshape(KX, 128, G4))
    wh = np.ascontiguousarray(Wcat[DIN:].reshape(KD, 128, G4))
    wx_dev, wh_dev = rt["bcast_fn"](
        jax.device_put(wx, rt["shard"]), jax.device_put(wh, rt["shard"])
    )

    biasrow = bias.reshape(1, G4)
    inj = np.zeros((BL + 1, BL), dtype=BF16)
    inj[:BL, :BL] = np.eye(BL, dtype=BF16)
    inj[BL, :] = BF16(1.0)
    ident = np.eye(BL, dtype=np.float32)
    id128 = np.eye(128, dtype=BF16)
    small = {
        "biasrow": jax.device_put(_rep8(biasrow), rt["shard"]),
        "inj": jax.device_put(_rep8(inj), rt["shard"]),
        "ident": jax.device_put(_rep8(ident), rt["shard"]),
        "id128": jax.device_put(_rep8(id128), rt["shard"]),
    }
    st = {"key": key, "wx": wx_dev, "wh": wh_dev, **small}
    _CACHE["weights"] = st
    return st


def kernel(batch, lengths, c0, W_i_w, W_i_b, W_f_w, W_f_b, W_c_w, W_c_b,
           W_o_w, W_o_b):
    rt = _runtime()
    jax = rt["jax"]

    st = _stage_weights(rt, W_i_w, W_i_b, W_f_w, W_f_b, W_c_w, W_c_b,
                        W_o_w, W_o_b)

    batch = np.asarray(batch)
    lengths = np.asarray(lengths, np.int32)
    c0 = np.asarray(c0, np.float32)

    # X: global [8*TBL, DIN] bf16; core c rows = batch[:, c*BL:(c+1)*BL, :]
    # flattened (t, bl). Single fused cast+copy pass.
    gx = np.empty((NCORES * TBL, DIN), BF16)
    gxv = gx.reshape(NCORES, T, BL, DIN)
    for c in range(NCORES):
        gxv[c] = batch[:, c * BL : (c + 1) * BL, :]
    x_dev = jax.device_put(gx, rt["shard"])

    # mask[b, t] = 1.0 where t < len(b)
    gmask = (
        np.arange(T, dtype=np.int32)[None, :] < lengths[:, None]
    ).astype(np.float32)  # [B, T], batch-major == core-major blocks of BL
    mask_dev = jax.device_put(gmask, rt["shard"])

    c_init = np.broadcast_to(c0, (BL, D))
    hcinit = np.stack([np.tanh(c_init), c_init]).astype(np.float32)
    hc_dev = jax.device_put(_rep8(hcinit), rt["shard"])

    zeros = rt["zeros_fn"]()

    by_name = {
        "x": x_dev,
        "wx": st["wx"],
        "wh": st["wh"],
        "biasrow": st["biasrow"],
        "inj": st["inj"],
        "ident": st["ident"],
        "id128": st["id128"],
        "mask": mask_dev,
        "hcinit": hc_dev,
    }
    args = [by_name[n] for n in rt["in_names"]] + list(zeros)
    out_arrs = rt["sharded"](*args)
    out16 = np.asarray(out_arrs[rt["out_names"].index("out")])

    # [8*TBL, D] f16 -> [T, B, D] f32, fused cast+copy
    out = np.empty((T, B, D), np.float32)
    o16v = out16.reshape(NCORES, T, BL, D)
    for c in range(NCORES):
        out[:, c * BL : (c + 1) * BL, :] = o16v[c]
    return out


if __name__ == "__main__":
    rng = np.random.default_rng(0)
    ins = {
        "batch": rng.standard_normal((T, B, DIN), dtype=np.float32),
        "lengths": rng.integers(0, T, size=(B,)).astype(np.int32),
        "c0": np.zeros((D,), np.float32),
    }
    for n in ["i", "f", "c", "o"]:
        ins[f"W_{n}_w"] = rng.standard_normal((DIN + D, D), dtype=np.float32) * 0.02
        ins[f"W_{n}_b"] = rng.standard_normal((D,), dtype=np.float32) * 0.02
    import time

    out = kernel(**ins)
    print(out.shape, out.dtype, np.abs(out).max())
    t0 = time.time()
    out = kernel(**ins)
    print(f"cached call: {time.time()-t0:.2f}s")


# revision 6
# speedup vs baseline: 1.6495x; 1.6495x over previous
"""Trainium2 Bass kernel for a ragged-sequence LSTM (nn_CH_LSTM).

Problem (hardcoded): T=512, B=64, DIN=1024, D=1024.
  c_init = broadcast(c0); h_init = tanh(c_init)
  per step t:  x = [x_t, h];  i,f,g,o = acts(x @ W_* + b_*)
               c = f*c + i*g;  h = o*tanh(c);  h[t >= len] = 0
  output: all h, [T, B, D] f32.

Device strategy (8 NeuronCores, data-parallel over batch: 8 seqs/core):
  phase 1: xw = X @ W_x as one large matmul; X arrives row-major bf16 and is
           transposed on-chip via the PE (identity matmul); xw -> DRAM bf16.
  phase 2: 512 sequential steps; per gate, xw_t + bias injected into PSUM via
           a tiny identity matmul, then h @ W_h accumulated with h^T-stationary
           matmuls (W_h resident in SBUF). State h/c fp32; output stored f16.

Wall-clock strategy (the axon tunnel moves ~50-80 MB/s, so bytes dominate):
  * sequences are sorted by length into core groups; only rows t < Lmax(core)
    are live (~56% for uniform lengths).  X goes up PACKED (live rows only,
    bf16) and is expanded on device via an index gather; the output comes
    back PACKED (f16) via an index gather.  Index arrays are data, so the
    expand/compact programs specialize only on the packed row count, padded
    to a 4096-row grid (<= 8 shapes ever).
  * weights upload once: 16MB sharded + on-fabric all_gather broadcast.
  * output zero-buffers are created on device (donated), never uploaded.
  * host passes are single-pass fused cast+copy (container has 1 CPU).
"""

import os
import sys

if "/opt/trn_rl_repo" not in sys.path:
    sys.path.insert(0, "/opt/trn_rl_repo")

import numpy as np
import ml_dtypes

T, B, DIN, D = 512, 64, 1024, 1024
NCORES = 8
BL = B // NCORES          # 8 sequences per core
G4 = 4 * D                # 4096 gate columns, order [i, g, f, o]
KD = D // 128             # 8 contraction tiles for the recurrent matmul
KX = DIN // 128           # 8 contraction tiles for the x matmul
TBL = T * BL              # 4096 flattened (t, b) rows per core
MT = TBL // 128           # 32 row tiles in phase 1
BF16 = ml_dtypes.bfloat16

_CACHE = {}


def _build_bass():
    import concourse.bass as bass
    import concourse.bacc as bacc
    import concourse.mybir as mybir
    from concourse import tile

    fp32 = mybir.dt.float32
    bf16 = mybir.dt.bfloat16
    f16 = mybir.dt.float16
    AF = mybir.ActivationFunctionType
    ALU = mybir.AluOpType
    ds = bass.ds

    nc = bacc.Bacc(trn_type="TRN2")

    x_d = nc.dram_tensor("x", [MT * 128, DIN], bf16, kind="ExternalInput")
    wx_d = nc.dram_tensor("wx", [KX, 128, G4], bf16, kind="ExternalInput")
    wh_d = nc.dram_tensor("wh", [KD, 128, G4], bf16, kind="ExternalInput")
    bias_d = nc.dram_tensor("biasrow", [1, G4], bf16, kind="ExternalInput")
    inj_d = nc.dram_tensor("inj", [BL + 1, BL], bf16, kind="ExternalInput")
    id_d = nc.dram_tensor("ident", [BL, BL], fp32, kind="ExternalInput")
    id128_d = nc.dram_tensor("id128", [128, 128], bf16, kind="ExternalInput")
    mask_d = nc.dram_tensor("mask", [BL, T], fp32, kind="ExternalInput")
    hc_d = nc.dram_tensor("hcinit", [2, BL, D], fp32, kind="ExternalInput")
    xw_d = nc.dram_tensor("xwbuf", [TBL, G4], bf16, kind="Internal")
    out_d = nc.dram_tensor("out", [TBL, D], f16, kind="ExternalOutput")

    with tile.TileContext(nc) as tc:
        with (
            tc.tile_pool(name="w", bufs=1) as wpool,
            tc.tile_pool(name="state", bufs=1) as spool,
            tc.tile_pool(name="gates", bufs=1) as gpool,
            tc.tile_pool(name="xwb", bufs=1) as xwbpool,
            tc.tile_pool(name="misc", bufs=1) as mpool,
            tc.tile_pool(name="o16", bufs=2) as o16pool,
        ):
            w_sb = wpool.tile([128, KD * G4], bf16)        # Wx in ph1, Wh in ph2
            h_sb = spool.tile([BL, D], fp32, tag="h")
            c_sb = spool.tile([BL, D], fp32, tag="c")
            mask_sb = mpool.tile([BL, T], fp32, tag="mask")
            inj_sb = mpool.tile([BL + 1, BL], bf16, tag="inj")
            id_sb = mpool.tile([BL, BL], fp32, tag="id")
            id128_sb = mpool.tile([128, 128], bf16, tag="id128")
            bias_sb = mpool.tile([1, G4], bf16, tag="bias")
            hT_sb = mpool.tile([128, KD * BL], bf16, tag="hT")
            xwb_A = xwbpool.tile([BL + 1, G4], bf16, tag="xa")
            xwb_B = xwbpool.tile([BL + 1, G4], bf16, tag="xb")
            i_sb = gpool.tile([BL, D], fp32, tag="gi")
            g_sb = gpool.tile([BL, D], fp32, tag="gg")
            f_sb = gpool.tile([BL, D], fp32, tag="gf")
            o_sb = gpool.tile([BL, D], fp32, tag="go")
            ig_sb = gpool.tile([BL, D], fp32, tag="ig")
            tanh_sb = gpool.tile([BL, D], fp32, tag="tc")

            nc.sync.dma_start(mask_sb[:], mask_d[:])
            nc.sync.dma_start(inj_sb[:], inj_d[:])
            nc.sync.dma_start(id_sb[:], id_d[:])
            nc.sync.dma_start(id128_sb[:], id128_d[:])
            nc.sync.dma_start(bias_sb[:], bias_d[:])
            nc.sync.dma_start(h_sb[:], hc_d[0])
            nc.sync.dma_start(c_sb[:], hc_d[1])
            nc.sync.dma_start(xwb_A[BL : BL + 1, :], bias_d[:])
            nc.sync.dma_start(xwb_B[BL : BL + 1, :], bias_d[:])
            for k in range(KX):
                nc.sync.dma_start(w_sb[:, k * G4 : (k + 1) * G4], wx_d[k])

            # ---------------- phase 1: xw = X @ Wx ----------------
            # X arrives row-major [row, din]; transpose 128x128 blocks on-chip.
            with (
                tc.tile_pool(name="ps1", bufs=2, space="PSUM") as ps1pool,
                tc.tile_pool(name="psT", bufs=1, space="PSUM") as psTpool,
                tc.tile_pool(name="xr", bufs=2) as xrpool,
                tc.tile_pool(name="xT", bufs=2) as xTpool,
                tc.tile_pool(name="xwo", bufs=3) as xwopool,
            ):
                with tc.For_i(0, MT, 1) as mv:
                    xr_sb = xrpool.tile([128, DIN], bf16, tag="xr")
                    nc.sync.dma_start(xr_sb[:], x_d[ds(mv * 128, 128), :])
                    psT = psTpool.tile([128, KX * 128], bf16, tag="psT")
                    for k in range(KX):
                        nc.tensor.transpose(
                            psT[:, k * 128 : (k + 1) * 128],
                            xr_sb[:, k * 128 : (k + 1) * 128],
                            id128_sb[:],
                        )
                    xT_sb = xTpool.tile([128, KX * 128], bf16, tag="xT")
                    nc.vector.tensor_copy(xT_sb[:], psT[:])
                    for q in range(4):
                        ps = ps1pool.tile([128, 1024], fp32, tag="ps")
                        for k in range(KX):
                            for n in range(2):
                                col = q * 1024 + n * 512
                                nc.tensor.matmul(
                                    ps[:, n * 512 : (n + 1) * 512],
                                    xT_sb[:, k * 128 : (k + 1) * 128],
                                    w_sb[:, k * G4 + col : k * G4 + col + 512],
                                    start=(k == 0),
                                    stop=(k == KX - 1),
                                )
                        xo = xwopool.tile([128, 1024], bf16, tag="xo")
                        nc.vector.tensor_copy(xo[:], ps[:])
                        nc.sync.dma_start(
                            xw_d[ds(mv * 128, 128), q * 1024 : (q + 1) * 1024],
                            xo[:],
                        )

            # ---------------- phase 2: recurrence ----------------
            for k in range(KD):
                nc.sync.dma_start(w_sb[:, k * G4 : (k + 1) * G4], wh_d[k])

            gate_specs = [
                (i_sb, AF.Sigmoid),
                (g_sb, AF.Tanh),
                (f_sb, AF.Sigmoid),
                (o_sb, AF.Sigmoid),
            ]

            with (
                tc.tile_pool(name="ps2", bufs=3, space="PSUM") as gps,
                tc.tile_pool(name="psT2", bufs=1, space="PSUM") as tps,
            ):
                def emit_step(t0, toff, xwb):
                    # h^T (bf16) for this step's stationary operands
                    hps = tps.tile([128, KD * BL], fp32, tag="ht")
                    for k in range(KD):
                        nc.tensor.transpose(
                            hps[:, k * BL : (k + 1) * BL],
                            h_sb[:, k * 128 : (k + 1) * 128],
                            id_sb[:],
                        )
                    nc.vector.tensor_copy(hT_sb[:], hps[:])

                    for gi, (gsb, func) in enumerate(gate_specs):
                        ps = gps.tile([BL, D], fp32, tag="g")
                        gcol = gi * D
                        for hh in range(2):
                            c0 = gcol + hh * 512
                            nc.tensor.matmul(
                                ps[:, hh * 512 : (hh + 1) * 512],
                                inj_sb[:],
                                xwb[:, c0 : c0 + 512],
                                start=True,
                                stop=False,
                            )
                        for k in range(KD):
                            for hh in range(2):
                                c0 = k * G4 + gcol + hh * 512
                                nc.tensor.matmul(
                                    ps[:, hh * 512 : (hh + 1) * 512],
                                    hT_sb[:, k * BL : (k + 1) * BL],
                                    w_sb[:, c0 : c0 + 512],
                                    start=False,
                                    stop=(k == KD - 1),
                                )
                        nc.scalar.activation(gsb[:], ps[:], func)

                    nc.vector.tensor_mul(ig_sb[:], i_sb[:], g_sb[:])
                    nc.vector.tensor_mul(c_sb[:], c_sb[:], f_sb[:])
                    nc.vector.tensor_add(c_sb[:], c_sb[:], ig_sb[:])
                    nc.scalar.activation(tanh_sb[:], c_sb[:], AF.Tanh)
                    tmask = mask_sb[:, ds(t0 + toff, 1)]
                    nc.vector.scalar_tensor_tensor(
                        h_sb[:], tanh_sb[:], tmask, o_sb[:],
                        ALU.mult, ALU.mult,
                    )
                    out16 = o16pool.tile([BL, D], f16, tag="o16")
                    nc.vector.tensor_copy(out16[:], h_sb[:])
                    nc.sync.dma_start(
                        out_d[ds(t0 * BL + toff * BL, BL), :], out16[:]
                    )

                with tc.For_i(0, T, 2) as t0:
                    nc.sync.dma_start(xwb_A[0:BL, :], xw_d[ds(t0 * BL, BL), :])
                    nc.sync.dma_start(
                        xwb_B[0:BL, :], xw_d[ds(t0 * BL + BL, BL), :]
                    )
                    emit_step(t0, 0, xwb_A)
                    emit_step(t0, 1, xwb_B)

    nc.finalize()
    return nc


def _runtime():
    """Build (once) the bass module + jitted device functions."""
    if "rt" in _CACHE:
        return _CACHE["rt"]

    import jax

    try:
        jax.config.update("jax_compilation_cache_dir", "/tmp/jax_cc_cache")
        jax.config.update("jax_persistent_cache_min_entry_size_bytes", -1)
        jax.config.update("jax_persistent_cache_min_compile_time_secs", 0)
    except Exception:
        pass

    import jax.numpy as jnp
    from jax.sharding import Mesh, PartitionSpec, NamedSharding
    from jax.experimental.shard_map import shard_map
    from concourse import mybir
    from concourse.bass2jax import (
        _bass_exec_p,
        install_neuronx_cc_hook,
        partition_id_tensor,
    )

    install_neuronx_cc_hook()
    nc = _build_bass()

    partition_name = (
        nc.partition_id_tensor.name if nc.partition_id_tensor else None
    )
    in_names, out_names, out_avals = [], [], []
    for alloc in nc.m.functions[0].allocations:
        if not isinstance(alloc, mybir.MemoryLocationSet):
            continue
        name = alloc.memorylocations[0].name
        if alloc.kind == "ExternalInput":
            if name != partition_name:
                in_names.append(name)
        elif alloc.kind == "ExternalOutput":
            out_names.append(name)
            shape = tuple(alloc.tensor_shape)
            dtype = mybir.dt.np(alloc.dtype)
            out_avals.append(jax.core.ShapedArray(shape, dtype))
    n_params = len(in_names)
    n_outs = len(out_avals)
    all_in_names = in_names + out_names
    if partition_name is not None:
        all_in_names.append(partition_name)

    def _body(*args):
        operands = list(args)
        if partition_name is not None:
            operands.append(partition_id_tensor())
        outs = _bass_exec_p.bind(
            *operands,
            out_avals=tuple(out_avals),
            in_names=tuple(all_in_names),
            out_names=tuple(out_names),
            lowering_input_output_aliases=(),
            sim_require_finite=True,
            sim_require_nnan=True,
            nc=nc,
        )
        return tuple(outs)

    devices = jax.devices()[:NCORES]
    mesh = Mesh(np.asarray(devices), ("core",))
    P = PartitionSpec
    shard = NamedSharding(mesh, P("core"))
    donate = tuple(range(n_params, n_params + n_outs))
    sharded = jax.jit(
        shard_map(
            _body,
            mesh=mesh,
            in_specs=(P("core"),) * (n_params + n_outs),
            out_specs=(P("core"),) * n_outs,
            check_rep=False,
        ),
        donate_argnums=donate,
        keep_unused=True,
    )

    zeros_fn = jax.jit(
        lambda: tuple(
            jnp.zeros((NCORES * a.shape[0], *a.shape[1:]), a.dtype)
            for a in out_avals
        ),
        out_shardings=tuple(shard for _ in out_avals),
    )

    # on-fabric broadcast: upload [n,128,G4] sharded, all_gather -> each core
    # holds the full array; global layout [8n,128,G4] with shard = full array.
    bcast_fn = jax.jit(
        shard_map(
            lambda a, b: (
                jax.lax.all_gather(a, "core", axis=0, tiled=True),
                jax.lax.all_gather(b, "core", axis=0, tiled=True),
            ),
            mesh=mesh,
            in_specs=(P("core"), P("core")),
            out_specs=(P("core"), P("core")),
        )
    )

    rt = {
        "jax": jax,
        "nc": nc,
        "in_names": in_names,
        "out_names": out_names,
        "sharded": sharded,
        "zeros_fn": zeros_fn,
        "bcast_fn": bcast_fn,
        "shard": shard,
        "mesh": mesh,
        "P": P,
        "shard_map": shard_map,
        "jnp": jnp,
    }
    _CACHE["rt"] = rt
    return rt


def _gather_fns(rt):
    """jitted expand (packed x -> full x) / compact (full out -> packed out).
    Index arrays are runtime data; jits specialize only on the packed shape."""
    if "gather_fns" in _CACHE:
        return _CACHE["gather_fns"]
    jax = rt["jax"]
    jnp = rt["jnp"]
    P = rt["P"]

    def expand_local(p_local, idx_local):
        full = jax.lax.all_gather(p_local, "core", axis=0, tiled=True)
        return jnp.take(full, idx_local, axis=0)

    def compact_local(g_local, idx_local):
        full = jax.lax.all_gather(g_local, "core", axis=0, tiled=True)
        return jnp.take(full, idx_local, axis=0)

    expand_fn = jax.jit(
        rt["shard_map"](expand_local, mesh=rt["mesh"],
                        in_specs=(P("core"), P("core")),
                        out_specs=P("core"), check_rep=False))
    compact_fn = jax.jit(
        rt["shard_map"](compact_local, mesh=rt["mesh"],
                        in_specs=(P("core"), P("core")),
                        out_specs=P("core"), check_rep=False))
    _CACHE["gather_fns"] = (expand_fn, compact_fn)
    return _CACHE["gather_fns"]


def _rep8(arr):
    """Host-replicate a small array to global [8*s0, ...]."""
    return np.ascontiguousarray(
        np.broadcast_to(arr[None], (NCORES,) + arr.shape).reshape(
            (NCORES * arr.shape[0],) + arr.shape[1:]
        )
    )


def _stage_weights(rt, W_i_w, W_i_b, W_f_w, W_f_b, W_c_w, W_c_b, W_o_w, W_o_b):
    key = tuple(id(a) for a in (W_i_w, W_i_b, W_f_w, W_f_b, W_c_w, W_c_b,
                                W_o_w, W_o_b))
    st = _CACHE.get("weights")
    if st is not None and st["key"] == key:
        return st
    jax = rt["jax"]
    # gate order [i, g, f, o]
    Wcat = np.concatenate([W_i_w, W_c_w, W_f_w, W_o_w], axis=1).astype(BF16)
    bias = np.concatenate([W_i_b, W_c_b, W_f_b, W_o_b]).astype(BF16)
    wx = np.ascontiguousarray(Wcat[:DIN].reshape(KX, 128, G4))
    wh = np.ascontiguousarray(Wcat[DIN:].reshape(KD, 128, G4))
    wx_dev, wh_dev = rt["bcast_fn"](
        jax.device_put(wx, rt["shard"]), jax.device_put(wh, rt["shard"])
    )

    biasrow = bias.reshape(1, G4)
    inj = np.zeros((BL + 1, BL), dtype=BF16)
    inj[:BL, :BL] = np.eye(BL, dtype=BF16)
    inj[BL, :] = BF16(1.0)
    ident = np.eye(BL, dtype=np.float32)
    id128 = np.eye(128, dtype=BF16)
    small = {
        "biasrow": jax.device_put(_rep8(biasrow), rt["shard"]),
        "inj": jax.device_put(_rep8(inj), rt["shard"]),
        "ident": jax.device_put(_rep8(ident), rt["shard"]),
        "id128": jax.device_put(_rep8(id128), rt["shard"]),
    }
    st = {"key": key, "wx": wx_dev, "wh": wh_dev, **small}
    _CACHE["weights"] = st
    return st


def kernel(batch, lengths, c0, W_i_w, W_i_b, W_f_w, W_f_b, W_c_w, W_c_b,
           W_o_w, W_o_b):
    rt = _runtime()
    jax = rt["jax"]
    expand_fn, compact_fn = _gather_fns(rt)

    st = _stage_weights(rt, W_i_w, W_i_b, W_f_w, W_f_b, W_c_w, W_c_b,
                        W_o_w, W_o_b)

    batch = np.asarray(batch)
    lengths = np.asarray(lengths, np.int32)
    c0 = np.asarray(c0, np.float32)

    # sort sequences by length into core groups; core c's live rows are
    # t < Lmax[c] (the group max), flattened (t, bl).
    perm = np.argsort(lengths, kind="stable")
    plen = lengths[perm]
    lmax = [int(plen[c * BL + BL - 1]) for c in range(NCORES)]
    r = [BL * m for m in lmax]
    off = np.cumsum([0] + r)
    rtot = int(off[-1])
    rp = max(4096, ((rtot + 4095) // 4096) * 4096)  # 4096-row shape grid

    # packed X: live rows only, bf16, single fused cast+copy pass
    gxp = np.empty((rp, DIN), BF16)
    for c in range(NCORES):
        if r[c]:
            gxp[off[c] : off[c] + r[c]].reshape(lmax[c], BL, DIN)[:] = (
                batch[: lmax[c], perm[c * BL : (c + 1) * BL], :]
            )
    gxp[rtot:] = 0

    # expand index: full row (c, t, bl) <- packed row; dead rows read row 0
    # (their outputs are masked to zero on device and never fetched).
    eidx = np.zeros((NCORES * TBL,), np.int32)
    cidx = np.zeros((rp,), np.int32)
    for c in range(NCORES):
        if r[c]:
            ar = np.arange(r[c], dtype=np.int32)
            eidx[c * TBL : c * TBL + r[c]] = off[c] + ar
            cidx[off[c] : off[c] + r[c]] = c * TBL + ar

    xp_dev = jax.device_put(gxp, rt["shard"])
    eidx_dev = jax.device_put(eidx, rt["shard"])
    cidx_dev = jax.device_put(cidx, rt["shard"])
    x_dev = expand_fn(xp_dev, eidx_dev)

    # mask[b, t] = 1.0 where t < len(b), rows in permuted (core-group) order
    gmask = (
        np.arange(T, dtype=np.int32)[None, :] < plen[:, None]
    ).astype(np.float32)
    mask_dev = jax.device_put(gmask, rt["shard"])

    c_init = np.broadcast_to(c0, (BL, D))
    hcinit = np.stack([np.tanh(c_init), c_init]).astype(np.float32)
    hc_dev = jax.device_put(_rep8(hcinit), rt["shard"])

    zeros = rt["zeros_fn"]()

    by_name = {
        "x": x_dev,
        "wx": st["wx"],
        "wh": st["wh"],
        "biasrow": st["biasrow"],
        "inj": st["inj"],
        "ident": st["ident"],
        "id128": st["id128"],
        "mask": mask_dev,
        "hcinit": hc_dev,
    }
    args = [by_name[n] for n in rt["in_names"]] + list(zeros)
    out_arrs = rt["sharded"](*args)
    outp = np.asarray(
        compact_fn(out_arrs[rt["out_names"].index("out")], cidx_dev)
    )

    # packed f16 -> [T, B, D] f32 (calloc'd zeros cover t >= len rows)
    out = np.zeros((T, B, D), np.float32)
    for c in range(NCORES):
        if r[c]:
            out[: lmax[c], perm[c * BL : (c + 1) * BL], :] = (
                outp[off[c] : off[c] + r[c]].reshape(lmax[c], BL, D)
            )
    return out


if __name__ == "__main__":
    rng = np.random.default_rng(0)
    ins = {
        "batch": rng.standard_normal((T, B, DIN), dtype=np.float32),
        "lengths": rng.integers(0, T, size=(B,)).astype(np.int32),
        "c0": np.zeros((D,), np.float32),
    }
    for n in ["i", "f", "c", "o"]:
        ins[f"W_{n}_w"] = rng.standard_normal((DIN + D, D), dtype=np.float32) * 0.02
        ins[f"W_{n}_b"] = rng.standard_normal((D,), dtype=np.float32) * 0.02
    import time

    out = kernel(**ins)
    print(out.shape, out.dtype, np.abs(out).max())
    t0 = time.time()
    out = kernel(**ins)
    print(f"cached call: {time.time()-t0:.2f}s")
